# revision 14
# baseline (speedup 1.0000x reference)
"""Trainium2 Bass kernel for DiffMultiHeadedAttention (differential attention).

Model (per reference):
    q = x @ Wq.T + bq; k = ef @ Wk.T + bk; v = ef @ Wv.T + bv
    lambda_full = exp(sum(lq1*lk1)) - exp(sum(lq2*lk2)) + 0.8
    att  = softmax(causal_mask(q_hh @ k_hh.T / sqrt(32)))   per 32 half-heads
    out_h = (att[2h] - lambda_full * att[2h+1]) @ v_h       per 16 heads
B=4, T=N=1024, H=16 heads of 64, 2H=32 half-heads of 32.

Sharding over 8 cores: core c = (batch b = c//2, head-group hg = c%2).
Each core owns one batch element and 8 full heads (16 half-heads) and
computes out^T [512, 1024] (fp16); the host casts/transposes/reassembles.

v2 design (from trace analysis of the v1 kernel @192us):
  - QK is emitted per (oc, tcv) slot with all four (j, s) half-head
    matmuls at PE row positions 0/32/64/96 -> 4 concurrent K=32 streams
    (row tiling), instead of 2.
  - Softmax combine: the old path burned ~60us of DVE/GpSimd on
    1-partition tiles (Sh copy, reciprocal, lambda-mul) plus a PSUM->SBUF
    copy and an accumulating output DMA.  New path per (h, tcv):
    gpsimd broadcast of the PSUM denominator row -> [64,512] SBUF,
    partition-aligned reciprocal, multiply straight out of PSUM with the
    -lambda fold fused via scalar_tensor_tensor, fp16 add of the two
    half-head terms, one fp16 output DMA.
  - Schedule: k0/q0 projection chains run first so the scalar engine's
    exp stream (~80us, the per-slot critical engine) starts immediately;
    v-projection and later-oc q/k chains are fillers interleaved between
    QK groups and AV sweeps with hand-assigned deadlines.
  - PSUM: pj 2 banks + qk 4 (2 x [128,2,512]) + av 2 (2 x [65,512]) = 8.
"""

import math

import numpy as np

B, T, N, HIDDEN = 4, 1024, 1024, 1024
H, HEAD, HALF = 16, 64, 32
O = 512            # per-core hidden slice (8 heads * 64)
HPC = 8            # heads per core
LAMBDA_INIT = 0.8
SCALE = 1.0 / math.sqrt(HALF)
P = 128
IC = HIDDEN // P   # 8 contraction chunks
OC = O // P        # 4 output chunks of the projections
NT = N // P        # 8 n-tiles (keys)
NCORES = 8

_STATE = {}


def _build_nc():
    from contextlib import ExitStack

    import concourse.bacc as bacc
    import concourse.mybir as mybir
    import concourse.tile as tile
    from concourse.bass import ts

    f32 = mybir.dt.float32
    f16 = mybir.dt.float16
    AF = mybir.ActivationFunctionType
    ALU = mybir.AluOpType

    nc = bacc.Bacc("TRN2", target_bir_lowering=False, debug=False)

    xt_d = nc.dram_tensor("xt", [HIDDEN, T], f16, kind="ExternalInput")
    eft_d = nc.dram_tensor("eft", [HIDDEN, N], f16, kind="ExternalInput")
    wqt_d = nc.dram_tensor("wqt", [HIDDEN, O], f16, kind="ExternalInput")
    wkt_d = nc.dram_tensor("wkt", [HIDDEN, O], f16, kind="ExternalInput")
    wvt_d = nc.dram_tensor("wvt", [HIDDEN, O], f16, kind="ExternalInput")
    bq_d = nc.dram_tensor("bq", [1, O], f32, kind="ExternalInput")
    bk_d = nc.dram_tensor("bk", [1, O], f32, kind="ExternalInput")
    bv_d = nc.dram_tensor("bv", [1, O], f32, kind="ExternalInput")
    lq1_d = nc.dram_tensor("lq1", [1, HALF], f32, kind="ExternalInput")
    lq2_d = nc.dram_tensor("lq2", [1, HALF], f32, kind="ExternalInput")
    lk1_d = nc.dram_tensor("lk1", [1, HALF], f32, kind="ExternalInput")
    lk2_d = nc.dram_tensor("lk2", [1, HALF], f32, kind="ExternalInput")
    outT_d = nc.dram_tensor("outT", [O, T], f16, kind="ExternalOutput")

    with tile.TileContext(nc) as tc:
        with ExitStack() as ctx:
            const = ctx.enter_context(tc.tile_pool(name="const", bufs=1))

            # ---- input loads: per-ic DMAs in priority waves, triggers spread
            # over four sequencers so descriptor generation parallelizes ----
            big = ctx.enter_context(tc.tile_pool(name="big", bufs=1))
            efT = big.tile([P, IC, N], f16)
            wkT = big.tile([P, IC, O], f16)
            xT = big.tile([P, IC, T], f16)
            wqT = big.tile([P, IC, O], f16)
            wvT = big.tile([P, IC, O], f16)

            # wave 1: weights for the k0/q0 chains (small, 1MB each)
            for ic in range(IC):
                nc.sync.dma_start(wkT[:, ic, :], wkt_d[ts(ic, P), :])
                nc.scalar.dma_start(wqT[:, ic, :], wqt_d[ts(ic, P), :])
            # wave 2: activations (2MB each)
            for ic in range(IC):
                nc.sync.dma_start(efT[:, ic, :], eft_d[ts(ic, P), :])
                nc.scalar.dma_start(xT[:, ic, :], xt_d[ts(ic, P), :])
            # wave 3: v weights (needed by fills from slot 0 on)
            for ic in range(IC):
                nc.gpsimd.dma_start(wvT[:, ic, :], wvt_d[ts(ic, P), :])

            # ---- lambda_full (tiny, computed once) ----
            lam_in = const.tile([1, 4, HALF], f32)
            nc.sync.dma_start(lam_in[:, 0, :], lq1_d[:])
            nc.sync.dma_start(lam_in[:, 1, :], lk1_d[:])
            nc.sync.dma_start(lam_in[:, 2, :], lq2_d[:])
            nc.sync.dma_start(lam_in[:, 3, :], lk2_d[:])
            lam_tmp = const.tile([1, 2, HALF], f32)
            nc.vector.tensor_mul(lam_tmp[:, 0, :], lam_in[:, 0, :], lam_in[:, 1, :])
            nc.vector.tensor_mul(lam_tmp[:, 1, :], lam_in[:, 2, :], lam_in[:, 3, :])
            lam_s = const.tile([1, 2], f32)
            nc.vector.tensor_reduce(
                lam_s, lam_tmp, axis=mybir.AxisListType.X, op=ALU.add
            )
            lam_e = const.tile([1, 2], f32)
            nc.scalar.activation(lam_e, lam_s, AF.Exp)
            # lam_neg = -(e1 - e2 + 0.8) = e2 - e1 - 0.8
            lam_neg = const.tile([1, 1], f32)
            nc.vector.tensor_sub(lam_neg, lam_e[:, 1:2], lam_e[:, 0:1])
            nc.vector.tensor_scalar_add(lam_neg, lam_neg, -LAMBDA_INIT)
            lam_neg64 = const.tile([64, 1], f32)
            nc.gpsimd.partition_broadcast(lam_neg64, lam_neg)

            # 0/1 upper-triangular mask (keep t_local >= n_local), doubled
            # along a middle dim so one DVE mul masks both half-heads.
            tri2 = const.tile([P, 2, P], f16)
            neg3 = const.tile([P, 1], f32)
            nc.vector.memset(neg3, -3.0)
            nc.gpsimd.memset(tri2, 1.0)
            nc.gpsimd.affine_select(
                out=tri2,
                in_=tri2,
                compare_op=ALU.is_ge,
                fill=0.0,
                base=0,
                pattern=[[0, 2], [1, P]],
                channel_multiplier=-1,
            )

            # ---- biases ----
            bq_sb = const.tile([P, OC], f32)
            nc.sync.dma_start(bq_sb, bq_d[0].rearrange("(a p) -> p a", p=P))
            bk_sb = const.tile([P, OC], f32)
            nc.sync.dma_start(bk_sb, bk_d[0].rearrange("(a p) -> p a", p=P))
            bv_1 = const.tile([1, O], f32)
            nc.sync.dma_start(bv_1, bv_d[:])
            bvb = const.tile([P, O], f32)
            nc.gpsimd.partition_broadcast(bvb, bv_1)

            # ---- persistent projection outputs ----
            proj = ctx.enter_context(tc.tile_pool(name="proj", bufs=1))
            qT = proj.tile([P, OC, T], f16)          # [d-part, oc, t]
            kT = proj.tile([P, OC, N], f16)          # [d-part, oc, n]
            # [n-part, nt, h, v(64) | 1]: the ones column makes the AV matmul
            # emit the softmax denominator on PSUM partition 64.
            vaug = proj.tile([P, NT, HPC, HEAD + 1], f16)
            nc.vector.memset(vaug[:, :, :, HEAD : HEAD + 1], 1.0)

            # ---- PSUM pools: qk tag [128,2,512] x2 bufs (4 banks, shared by
            # QK groups and projection chains) + av tag [65,2,512] x2 (4) ----
            ps_qk = ctx.enter_context(
                tc.tile_pool(name="ps_qk", bufs=2, space="PSUM")
            )
            ps_av = ctx.enter_context(
                tc.tile_pool(name="ps_av", bufs=2, space="PSUM")
            )

            att_sb = ctx.enter_context(tc.tile_pool(name="att_sb", bufs=4))

            # ---------- PE work units (fillers) ----------
            def v_unit(nt_):
                def emit():
                    psv = ps_qk.tile([P, 2, 512], f32, tag="qk", name="psv")[:, 0, :]
                    for ic in range(IC):
                        nc.tensor.matmul(
                            psv,
                            efT[:, ic, ts(nt_, P)],
                            wvT[:, ic, :],
                            start=(ic == 0),
                            stop=(ic == IC - 1),
                        )
                    nc.vector.tensor_add(
                        vaug[:, nt_, :, 0:HEAD],
                        psv[:].rearrange("p (h d) -> p h d", h=HPC),
                        bvb[:].rearrange("p (h d) -> p h d", h=HPC),
                    )

                return emit

            def chain_unit(which, oc, t2):
                wT, b_sb, actT, dstT = (
                    (wkT, bk_sb, efT, kT) if which == "k" else (wqT, bq_sb, xT, qT)
                )

                def emit():
                    psj = ps_qk.tile([P, 2, 512], f32, tag="qk", name="psj")[:, 0, :]
                    for ic in range(IC):
                        nc.tensor.matmul(
                            psj,
                            wT[:, ic, ts(oc, P)],
                            actT[:, ic, ts(t2, 512)],
                            start=(ic == 0),
                            stop=(ic == IC - 1),
                        )
                    nc.vector.tensor_scalar_add(
                        dstT[:, oc, ts(t2, 512)], psj, b_sb[:, oc : oc + 1]
                    )

                return emit

            # E tiles saved per (oc, tcv, nt, j) for the lagged AV sweeps
            Es = {}

            def widths(tcv):
                # per n-tile: (nt, cs, w) of the causally-needed t-span
                out = []
                nis = range(4) if tcv == 0 else range(NT)
                for nt_ in nis:
                    t0 = nt_ * P
                    cs = max(t0, 512 * tcv)
                    w = 512 * (tcv + 1) - cs
                    out.append((nt_, cs, w))
                return out

            def av_unit(h, tcv):
                """Both s-sweeps + combine for one (head, t-chunk)."""
                oc, j = h // 2, h % 2

                def emit():
                    wlist = widths(tcv)
                    last = wlist[-1][0]
                    avp = ps_av.tile([HEAD + 1, 2, 512], f32, tag="av", name="avp")
                    for s in range(2):
                        for nt_, cs, w in wlist:
                            E = Es[(oc, tcv, nt_, j)]
                            off = 512 - w
                            nc.tensor.matmul(
                                avp[:, s, off : off + w],
                                vaug[:, nt_, h, :],
                                E[:, s, :w],
                                start=(nt_ == 0),
                                stop=(nt_ == last),
                            )
                    # combine: m = P0/S0 - lambda*P1/S1, fp16 out.  S sits on
                    # PSUM partition 64; copy it down to partition 0 (DVE),
                    # reciprocal, gpsimd-broadcast to 64 partitions, then
                    # multiply straight out of PSUM.
                    sc = att_sb.tile([1, 2, 512], f32, tag="sc", bufs=3, name="sc")
                    nc.vector.tensor_copy(sc, avp[HEAD : HEAD + 1, :, :])
                    r1 = att_sb.tile([1, 2, 512], f32, tag="r1", bufs=3, name="r1")
                    nc.vector.reciprocal_approx_fast(out=r1, in_=sc)
                    Rb = att_sb.tile([HEAD, 2, 512], f32, tag="rb", bufs=3, name="rb")
                    nc.gpsimd.partition_broadcast(Rb, r1)
                    m0 = att_sb.tile([HEAD, 512], f16, tag="m0", bufs=2, name="m0")
                    nc.vector.tensor_mul(m0, avp[0:HEAD, 0, :], Rb[:, 0, :])
                    m1 = att_sb.tile([HEAD, 512], f16, tag="m1", bufs=2, name="m1")
                    nc.vector.scalar_tensor_tensor(
                        out=m1,
                        in0=avp[0:HEAD, 1, :],
                        scalar=lam_neg64,
                        in1=Rb[:, 1, :],
                        op0=ALU.mult,
                        op1=ALU.mult,
                    )
                    mc = att_sb.tile([HEAD, 512], f16, tag="mc", bufs=2, name="mc")
                    nc.gpsimd.tensor_add(mc, m0, m1)
                    nc.sync.dma_start(
                        outT_d[HEAD * h : HEAD * (h + 1), ts(tcv, 512)], mc
                    )

                return emit

            # ---------- QK + exp for one (oc, tcv) slot ----------
            def emit_qk_group(oc, tcv, nt_, cs, w):
                attps = []
                for j in range(2):
                    ps = ps_qk.tile([P, 2, 512], f32, tag="qk", name="attps")
                    attps.append(ps)
                    for s in range(2):
                        base = 64 * j + 32 * s
                        nc.tensor.matmul(
                            ps[:, s, :w],
                            kT[base : base + 32, oc, ts(nt_, P)],
                            qT[base : base + 32, oc, cs : cs + w],
                            start=True,
                            stop=True,
                            tile_position=(96, 0) if base == 96 else None,
                        )
                for j in range(2):
                    if w > 256:
                        E = att_sb.tile([P, 2, 512], f16, tag="Eb", bufs=26, name="E")
                    else:
                        E = att_sb.tile([P, 2, 256], f16, tag="Es", bufs=12, name="E")
                    Es[(oc, tcv, nt_, j)] = E
                    nc.scalar.activation(
                        E[:, :, :w],
                        attps[j][:, :, :w],
                        AF.Exp,
                        bias=neg3[:, 0:1],
                        scale=SCALE,
                    )
                    if cs == nt_ * P:
                        # diagonal-block causal mask; split across DVE/gpsimd
                        eng = nc.vector if j == 0 else nc.gpsimd
                        eng.tensor_mul(E[:, :, 0:P], E[:, :, 0:P], tri2)

            # ---------- schedule ----------
            # pre-slot chains (needed by slot 0)
            chain_unit("k", 0, 0)()
            chain_unit("q", 0, 0)()

            SLOTS = [
                # (oc, tcv, [units interleaved between QK nt-groups])
                (0, 0, [chain_unit("k", 0, 1), chain_unit("q", 0, 1),
                        v_unit(0), v_unit(1)]),
                (0, 1, [v_unit(2), v_unit(3), chain_unit("k", 1, 0),
                        chain_unit("q", 1, 0), av_unit(0, 0), av_unit(1, 0)]),
                (1, 0, [chain_unit("q", 1, 1), chain_unit("k", 1, 1),
                        v_unit(4), v_unit(5)]),
                (1, 1, [v_unit(6), v_unit(7), av_unit(0, 1),
                        chain_unit("k", 2, 0), av_unit(2, 0),
                        chain_unit("q", 2, 0), av_unit(3, 0)]),
                (2, 0, [chain_unit("k", 2, 1), chain_unit("q", 2, 1),
                        av_unit(1, 1)]),
                (2, 1, [chain_unit("k", 3, 0), av_unit(2, 1),
                        chain_unit("k", 3, 1), av_unit(4, 0),
                        chain_unit("q", 3, 1), av_unit(5, 0)]),
                (3, 1, [chain_unit("q", 3, 0), av_unit(3, 1),
                        av_unit(4, 1), av_unit(5, 1)]),
                (3, 0, [av_unit(6, 1), av_unit(7, 1)]),
            ]
            TAIL = [av_unit(6, 0), av_unit(7, 0)]

            for oc, tcv, units in SLOTS:
                wlist = widths(tcv)
                ui = 0
                for gi, (nt_, cs, w) in enumerate(wlist):
                    emit_qk_group(oc, tcv, nt_, cs, w)
                    if ui < len(units):
                        units[ui]()
                        ui += 1
                while ui < len(units):
                    units[ui]()
                    ui += 1
            for u in TAIL:
                u()

    nc.compile()
    return nc


def _ensure_axon_hooks():
    """concourse's trace path imports antenv.axon_hooks, which this image
    lacks; provide it (registering the real ctypes NTFF hook when available)
    so BASS_TRACE=1 degrades gracefully instead of crashing."""
    import sys
    import types

    if "antenv.axon_hooks" in sys.modules:
        return
    try:
        import antenv.axon_hooks  # noqa: F401

        return
    except ImportError:
        pass
    mod = types.ModuleType("antenv.axon_hooks")
    mod._hook = None
    mod.set_axon_ntff_profile_hook = lambda h: setattr(mod, "_hook", h)
    mod.get_axon_ntff_profile_hook = lambda: mod._hook
    sys.modules["antenv.axon_hooks"] = mod
    import os

    if os.environ.get("KERNEL_TRACE") == "1":
        try:
            from trn_agent_boot.trn_boot import _ntff_profile_via_ctypes

            mod._hook = _ntff_profile_via_ctypes("/opt/axon/libaxon_pjrt.so")
        except Exception:
            pass


def _get_state():
    if "nc" not in _STATE:
        from concourse.bass_utils import run_bass_kernel_spmd

        _ensure_axon_hooks()
        _STATE["nc"] = _build_nc()
        _STATE["run"] = run_bass_kernel_spmd
    return _STATE


def kernel(**inputs):
    st = _get_state()

    def f32c(a):
        return np.ascontiguousarray(np.asarray(a, dtype=np.float32))

    x = np.asarray(inputs["x"], dtype=np.float32)
    ef = np.asarray(inputs["encoder_feature"], dtype=np.float32)
    Wq, bq = np.asarray(inputs["Wq"], np.float32), np.asarray(inputs["bq"], np.float32)
    Wk, bk = np.asarray(inputs["Wk"], np.float32), np.asarray(inputs["bk"], np.float32)
    Wv, bv = np.asarray(inputs["Wv"], np.float32), np.asarray(inputs["bv"], np.float32)
    lq1 = f32c(inputs["lambda_q1"]).reshape(1, HALF)
    lq2 = f32c(inputs["lambda_q2"]).reshape(1, HALF)
    lk1 = f32c(inputs["lambda_k1"]).reshape(1, HALF)
    lk2 = f32c(inputs["lambda_k2"]).reshape(1, HALF)

    in_maps = []
    for c in range(NCORES):
        b, hg = c // 2, c % 2
        sl = slice(hg * O, (hg + 1) * O)
        in_maps.append(
            {
                "xt": np.ascontiguousarray(x[b].T.astype(np.float16)),
                "eft": np.ascontiguousarray(ef[b].T.astype(np.float16)),
                "wqt": np.ascontiguousarray(Wq[sl].T.astype(np.float16)),
                "wkt": np.ascontiguousarray(Wk[sl].T.astype(np.float16)),
                "wvt": np.ascontiguousarray(Wv[sl].T.astype(np.float16)),
                "bq": f32c(bq[sl]).reshape(1, O),
                "bk": f32c(bk[sl]).reshape(1, O),
                "bv": f32c(bv[sl]).reshape(1, O),
                "lq1": lq1,
                "lq2": lq2,
                "lk1": lk1,
                "lk2": lk2,
            }
        )

    res = st["run"](st["nc"], in_maps, core_ids=list(range(NCORES)))
    _STATE["last_results"] = res

    out = np.empty((B, T, HIDDEN), dtype=np.float32)
    for c in range(NCORES):
        b, hg = c // 2, c % 2
        out[b, :, hg * O : (hg + 1) * O] = res.results[c]["outT"].T.astype(np.float32)
    return out
